# revision 44
# baseline (speedup 1.0000x reference)
"""Pairwise IoU (8192x8192) on 8 Trainium2 NeuronCores via Bass/Tile.

Strategy
--------
Boxes can only overlap when both their x- and y-ranges are within the
max box extent of each other, so most of the 8192x8192 IoU matrix is
exactly zero.  Two-level windowing exploits this: boxes1 rows are
x-sorted and split across 8 cores (1024 rows each); within a core, rows
are y-sorted, and the core's x-relevant boxes2 subset (the only columns
that can ever overlap its rows) is Y1-sorted.  Each 128-row i-tile then
only scores a contiguous window of ~1.2-2.6k y-overlapping columns
(~18% of M on average).  Provably-zero entries are assembled on the
host.

The device program is SPMD (one program, 8 cores), so per-tile window
offsets/widths are compile-time constants OFFS[t]/WT[t]; the host packs
each core's columns so that its tile-t window lies inside
[base_c + OFFS[t], base_c + OFFS[t] + WT[t]).  The (base, OFFS, WT)
decomposition is solved as an LP from the actual data at first call.

Per-core device kernel, per [128, WT] output tile:
  rx    = relu(min(x2_i, X2_j) - max(x1_i, X1_j))   custom DVE op (fp16 out)
  ry    = same for y                                custom DVE op (fp16 out)
  inter = rx*ry                                     DVE tensor_tensor (fp16, 2x)
  p     = a2e_j - inter                             TensorE (ones rank-1 +
                                                    (-I)@inter, fp16) -> PSUM
  rinv  = Exp(-Ln(p + a1_i))                        ScalarE LUTs (a1 via the
                                                    per-partition Ln bias)
  out   = inter * rinv                              DVE tensor_tensor (bf16 out)
"""

import numpy as np

N = 8192
M = 8192
NCORES = 8
ROWS = N // NCORES  # rows of boxes1 per core
P = 128  # partitions
NT = ROWS // P  # 8 i-tiles per core
PS = 512  # psum bank width (fp32)
EPS = 1e-7

# 1-Newton reciprocal constants (fused DVE division path)
RC0 = -0.23549792
RC1 = 2.0017324

_COMPILED = {}


def _np_recip1(u):
    nu = (~np.asarray(u, np.float32).view(np.int32)).view(np.float32)
    y0 = (nu * np.float32(RC0)).astype(np.float32)
    return (y0 * (np.float32(RC1) - u * y0)).astype(np.float32)


def _register_op(name, spec, subdim=False):
    import concourse.dve_ops as dve_ops
    from concourse.dve_spec import lower
    from concourse.dve_uop import DveOpSpec

    for op in dve_ops.OPS:
        if op.name == name:
            return op
    shas = {}
    for ver in ("v3", "v4"):
        try:
            shas[ver] = DveOpSpec(
                name=name, opcode=0, uops=lower(spec, ver=ver)
            ).sha(ver)
        except Exception:
            pass
    op = dve_ops.DveOp(name, spec, subdim=subdim, uops_sha=shas)
    dve_ops.OPS.append(op)
    dve_ops.CUSTOM_DVE_SPECS[op.name] = op.spec
    dve_ops._SUB_OPCODE_FOR_NAME[op.name] = (
        dve_ops._CUSTOM_DVE_ROW_BASE + len(dve_ops.OPS) - 1
    )
    return op


def _ensure_ops():
    """Register the IOU_EDGE and IOU_DIV custom DVE ops (idempotent)."""
    from concourse.dve_spec import (
        C0,
        C1,
        C2,
        AluOp,
        Bin,
        Spec,
        Src0,
        Src1,
        maxx,
        minn,
        relu,
    )

    edge = _register_op(
        "IOU_EDGE",
        Spec(
            body=relu(minn(Src1, C1) - maxx(Src0, C0)),
            reference=lambda in0, in1, s0, s1, imm2: np.maximum(
                np.minimum(in1, s1) - np.maximum(in0, s0), 0.0
            ).astype(np.float32),
        ),
    )
    # u = (a1 + a2e) - inter; out = inter * recip1NR(u).  8 ALU stages.
    _t1 = C0 + Src1
    _u = _t1 - Src0
    _nu = Bin(AluOp.BITWISE_NOT, _u, _u)
    _y0 = _nu * C1
    _y1 = _y0 * (C2 - _u * _y0)
    div = _register_op(
        "IOU_DIV",
        Spec(
            body=Src0 * _y1,
            reference=lambda in0, in1, s0, s1, imm2: (
                in0 * _np_recip1((s0 + in1) - in0)
            ).astype(np.float32),
        ),
    )
    return edge, div


def _build_program(WT, OFFS, WCOL):
    from contextlib import ExitStack

    import concourse.bacc as bacc
    import concourse.mybir as mybir
    import concourse.tile as tile

    iou_edge, iou_div = _ensure_ops()

    f32 = mybir.dt.float32
    f16 = mybir.dt.float16
    bf16 = mybir.dt.bfloat16
    act = mybir.ActivationFunctionType
    nc = bacc.Bacc(
        "TRN2",
        target_bir_lowering=False,
        debug=False,
        enable_asserts=False,
        num_devices=NCORES,
    )

    # The default act-table placement resolves Ln and Exp to different
    # table sets, reloading tables on every switch (~2.7us each).  Route
    # both to the one set that contains them, preserving set indices.
    import types

    import bass_rust as _bass_rust
    from concourse.hw_specs import get_activation_tables

    def _insert_act_table_loads(self):
        has_activation = any(
            isinstance(i, mybir.InstActivation)
            for b in self.main_func.blocks
            for i in b.instructions
        )
        if not has_activation:
            return
        both = {act.Ln, act.Exp}
        tables = [
            (name, fns if both <= fns else fns - both)
            for name, fns in get_activation_tables(self.m.arch).items()
        ]
        _bass_rust.insert_act_table_loads(self, tables)

    nc.insert_act_table_loads = types.MethodType(_insert_act_table_loads, nc)

    WMAX = max(WT)
    # Processing order: start at the narrowest slot, then greedily take the
    # slot needing the fewest not-yet-loaded columns (windows overlap, so
    # increments are small); keep the widest slot off the last two positions.
    s0 = min(range(NT), key=lambda t: WT[t])
    order = [s0]
    lo, hi = OFFS[s0], OFFS[s0] + WT[s0]
    rest = set(range(NT)) - {s0}
    while rest:
        t = min(
            rest,
            key=lambda t: (
                max(0, lo - OFFS[t]) + max(0, OFFS[t] + WT[t] - hi),
                WT[t],
            ),
        )
        order.append(t)
        rest.remove(t)
        lo = min(lo, OFFS[t])
        hi = max(hi, OFFS[t] + WT[t])
    wmax_slot = max(range(NT), key=lambda t: WT[t])
    if order.index(wmax_slot) >= NT - 2 and NT >= 3:
        order.remove(wmax_slot)
        order.insert(NT - 3, wmax_slot)
    # column-interval load chunks matching the final processing order
    chunks = []
    H0 = (WT[s0] // 2 + 31) & ~31
    chunks.append((OFFS[s0], OFFS[s0] + H0))
    chunks.append((OFFS[s0] + H0, OFFS[s0] + WT[s0]))
    lo, hi = OFFS[s0], OFFS[s0] + WT[s0]
    for t in order[1:]:
        if OFFS[t] < lo:
            chunks.append((OFFS[t], lo))
            lo = OFFS[t]
        if OFFS[t] + WT[t] > hi:
            chunks.append((hi, OFFS[t] + WT[t]))
            hi = OFFS[t] + WT[t]
    if 0 < lo:
        chunks.append((0, lo))
    if hi < WCOL:
        chunks.append((hi, WCOL))

    # DRAM I/O. boxes2 coord rows are host-replicated across partitions.
    x1b = nc.dram_tensor("x1b", [P, WCOL], f32, kind="ExternalInput").ap()
    x2b = nc.dram_tensor("x2b", [P, WCOL], f32, kind="ExternalInput").ap()
    y1b = nc.dram_tensor("y1b", [P, WCOL], f32, kind="ExternalInput").ap()
    y2b = nc.dram_tensor("y2b", [P, WCOL], f32, kind="ExternalInput").ap()
    # Per-partition scalars: for i-tile t, columns t*5+k hold
    # (x1, x2, y1, y2, area1) of sorted boxes1 row t*128+p.
    sc = nc.dram_tensor("sc", [P, NT * 5], f32, kind="ExternalInput").ap()
    a2e1 = nc.dram_tensor("a2e1", [1, WCOL], f16, kind="ExternalInput").ap()
    ones1 = nc.dram_tensor("ones1", [1, P], f16, kind="ExternalInput").ap()
    negi = nc.dram_tensor("negi", [P, P], f16, kind="ExternalInput").ap()
    out = nc.dram_tensor("out", [ROWS, WMAX], bf16, kind="ExternalOutput").ap()

    with tile.TileContext(nc) as tc, ExitStack() as ctx:
        bc = ctx.enter_context(tc.tile_pool(name="bc", bufs=1))
        scp = ctx.enter_context(tc.tile_pool(name="scp", bufs=1))
        work = ctx.enter_context(tc.tile_pool(name="work", bufs=3))
        outp = ctx.enter_context(tc.tile_pool(name="outp", bufs=3))
        psum = ctx.enter_context(tc.tile_pool(name="psum", bufs=1, space="PSUM"))

        sct = scp.tile([P, NT * 5], f32)
        x1t = bc.tile([P, WCOL], f32)
        x2t = bc.tile([P, WCOL], f32)
        y1t = bc.tile([P, WCOL], f32)
        y2t = bc.tile([P, WCOL], f32)
        a2e1t = scp.tile([1, WCOL], f16)
        ones1t = scp.tile([1, P], f16)
        negit = scp.tile([P, P], f16)

        # Coordinate loads in processing-order chunks, interleaved across the
        # two HWDGE queues (front-loading transfers minimizes the window in
        # which DMA writes contend with compute for SBUF ports).
        for ci, (a, b) in enumerate(chunks):
            nc.sync.dma_start(x1t[:, a:b], x1b[:, a:b])
            nc.scalar.dma_start(x2t[:, a:b], x2b[:, a:b])
            nc.sync.dma_start(y1t[:, a:b], y1b[:, a:b])
            nc.scalar.dma_start(y2t[:, a:b], y2b[:, a:b])
            if ci == 0:
                # sct is tiny (20KB): issue after the critical x/y heads so
                # its descriptor-gen doesn't delay them, still ready first.
                nc.sync.dma_start(sct[:], sc[:])
            if ci == 1:
                nc.scalar.dma_start(negit[:], negi[:])
                nc.scalar.dma_start(ones1t[:], ones1[:])
                nc.scalar.dma_start(a2e1t[:], a2e1[:])

        for ti, t in enumerate(order):
            o = OFFS[t]
            c = t * 5
            W = WT[t]
            rx = work.tile([P, WMAX], f16, tag="rx")
            ry = work.tile([P, WMAX], f16, tag="ry")
            inter = work.tile([P, WMAX], f16, tag="inter")
            ot = outp.tile([P, WMAX], bf16, tag="ot")

            # First tile: edge ops in interleaved x/y column chunks so each
            # starts as soon as its head DMA lands.
            echunks = [(0, H0), (H0, W)] if ti == 0 else [(0, W)]
            for e0, e1 in echunks:
                nc.vector._custom_dve(
                    iou_edge,
                    out=rx[:, e0:e1],
                    in0=x1t[:, o + e0 : o + e1],
                    in1=x2t[:, o + e0 : o + e1],
                    s0=sct[:, c : c + 1],
                    s1=sct[:, c + 1 : c + 2],
                )
                nc.vector._custom_dve(
                    iou_edge,
                    out=ry[:, e0:e1],
                    in0=y1t[:, o + e0 : o + e1],
                    in1=y2t[:, o + e0 : o + e1],
                    s0=sct[:, c + 2 : c + 3],
                    s1=sct[:, c + 3 : c + 4],
                )
            nc.vector.tensor_mul(inter[:, :W], rx[:, :W], ry[:, :W])
            NCH = -(-W // PS)


            ua = work.tile([P, WMAX], f32, tag="ua")
            rinv = work.tile([P, WMAX], f16, tag="rinv")
            # p = a2e - inter on TensorE (constant stationaries); matmuls
            # per 512-col bank, Ln over 2-bank chunks (fewer ScalarE ops);
            # a1 rides the Ln bias.
            pts = []
            for k in range(0, NCH, 2):
                c0 = k * PS
                c1 = min(W, c0 + 2 * PS)
                pt = psum.tile([P, 2 * PS], f32, tag="pt", bufs=4)
                pt = pt[:, : c1 - c0]
                pts.append((pt, c0, c1))
                for m0 in range(0, c1 - c0, PS):
                    m1 = min(c1 - c0, m0 + PS)
                    nc.tensor.matmul(
                        pt[:, m0:m1],
                        ones1t[:],
                        a2e1t[:, o + c0 + m0 : o + c0 + m1],
                        start=True,
                        stop=False,
                    )
            for pt, c0, c1 in pts:
                for m0 in range(0, c1 - c0, PS):
                    m1 = min(c1 - c0, m0 + PS)
                    nc.tensor.matmul(
                        pt[:, m0:m1],
                        negit[:],
                        inter[:, c0 + m0 : c0 + m1],
                        start=False,
                        stop=True,
                    )
            for pt, c0, c1 in pts:
                nc.scalar.activation(
                    ua[:, c0:c1], pt[:], act.Ln, bias=sct[:, c + 4 : c + 5]
                )

            # Last tile: split the output stage so the final store overlaps.
            if ti == NT - 1:
                h = (NCH // 2) * PS
                ochunks = [(0, min(h, W)), (min(h, W), W)]
                ochunks = [(a, b) for a, b in ochunks if b > a]
            else:
                ochunks = [(0, W)]
            outq = nc.sync if ti % 2 == 0 else nc.scalar
            for a, b in ochunks:
                nc.scalar.activation(
                    rinv[:, a:b], ua[:, a:b], act.Exp, scale=-1.0
                )
                nc.vector.tensor_mul(ot[:, a:b], inter[:, a:b], rinv[:, a:b])
                outq.dma_start(out[t * P : (t + 1) * P, a:b], ot[:, a:b])

    nc.compile()
    return nc


def _get_program(WT, OFFS, WCOL):
    key = (tuple(WT), tuple(OFFS), WCOL)
    if key not in _COMPILED:
        _COMPILED[key] = _build_program(list(WT), list(OFFS), WCOL)
    return _COMPILED[key]


def _plan(boxes1, boxes2):
    """Two-level windowing: rows are x-sorted into per-core bands; within
    each core, rows are y-sorted and each core's x-relevant boxes2 subset
    is Y1-sorted, so each 128-row i-tile needs only a contiguous window of
    y-overlapping columns.  Returns per-slot offsets/widths (compile-time)
    plus per-core packing data."""
    b1 = np.ascontiguousarray(boxes1, dtype=np.float32)
    b2 = np.ascontiguousarray(boxes2, dtype=np.float32)
    p1 = np.argsort(b1[:, 0], kind="stable")
    s1 = b1[p1]
    X1_2, Y1_2 = b2[:, 0], b2[:, 1]
    X2_2, Y2_2 = b2[:, 2], b2[:, 3]
    wmax2 = float((X2_2 - X1_2).max())
    hmax2 = float((Y2_2 - Y1_2).max())
    x1order = np.argsort(X1_2, kind="stable")
    X1s = X1_2[x1order]

    rowids = []  # per core: original boxes1 ids, y-sorted  [ROWS]
    colids = []  # per core: original boxes2 ids, Y1-sorted x-relevant subset
    jL = np.empty((NCORES, NT), np.int64)
    jR = np.empty((NCORES, NT), np.int64)
    for c in range(NCORES):
        blk = s1[c * ROWS : (c + 1) * ROWS]
        lo = np.searchsorted(X1s, np.float32(blk[:, 0].min() - wmax2) - 1e-3)
        hi = np.searchsorted(X1s, np.float32(blk[:, 2].max()) + 1e-3)
        cj = x1order[lo:hi]
        cj = cj[np.argsort(Y1_2[cj], kind="stable")]
        colids.append(cj)
        Y1c = Y1_2[cj]
        yord = np.argsort(blk[:, 1], kind="stable")
        rowids.append(p1[c * ROWS + yord])
        blky = blk[yord]
        for t in range(NT):
            rows = blky[t * P : (t + 1) * P]
            jL[c, t] = np.searchsorted(
                Y1c, np.float32(rows[:, 1].min() - hmax2) - 1e-3
            )
            jR[c, t] = np.searchsorted(Y1c, np.float32(rows[:, 3].max()) + 1e-3)

    # Decompose window starts into base_c + off_t minimizing total width:
    # an LP over (off_t, base_c, W_t) with containment constraints.
    offs = None
    try:
        from scipy.optimize import linprog

        nv = 2 * NT + NCORES
        A_ub, b_ub = [], []
        for c in range(NCORES):
            for t in range(NT):
                r1 = np.zeros(nv)
                r1[t] = 1
                r1[NT + c] = 1
                A_ub.append(r1)
                b_ub.append(jL[c, t])
                r2 = np.zeros(nv)
                r2[t] = -1
                r2[NT + c] = -1
                r2[NT + NCORES + t] = -1
                A_ub.append(r2)
                b_ub.append(-jR[c, t])
        cvec = np.zeros(nv)
        cvec[NT + NCORES :] = 1
        res = linprog(
            cvec,
            A_ub=np.array(A_ub),
            b_ub=np.array(b_ub),
            bounds=[(None, None)] * (NT + NCORES) + [(0, None)] * NT,
            method="highs",
        )
        if res.status == 0:
            offs = np.floor(res.x[:NT]).astype(np.int64)
    except Exception:
        pass
    if offs is None:
        offs = np.median(jL - jL[:, :1], axis=0).astype(np.int64)
    offs = 2 * ((offs - offs.min()) // 2)
    bases = (jL - offs[None, :]).min(axis=1)
    wt = (jR - offs[None, :] - bases[:, None]).max(axis=0)
    WT = [min(int(-(-max(int(w), 64) // 32) * 32), M + 512) for w in wt]
    WCOL = int(max(offs[t] + WT[t] for t in range(NT)))
    return dict(
        b1=b1, b2=b2, rowids=rowids, colids=colids,
        WT=WT, OFFS=[int(o) for o in offs], WCOL=WCOL, bases=bases,
    )


def _make_in_maps(plan):
    b1, b2 = plan["b1"], plan["b2"]
    WCOL, bases = plan["WCOL"], plan["bases"]

    a2e = (
        (b2[:, 2] - b2[:, 0]) * (b2[:, 3] - b2[:, 1]) + np.float32(EPS)
    ).astype(np.float32)

    in_maps = []
    for c in range(NCORES):
        cj = plan["colids"][c]
        idx = bases[c] + np.arange(WCOL)
        valid = (idx >= 0) & (idx < len(cj))
        idxc = cj[np.clip(idx, 0, len(cj) - 1)]
        pad = np.float32(-1e6)

        def rep(vec, fill, dt=np.float32):
            row = np.where(valid, vec[idxc], fill).astype(dt)
            return np.ascontiguousarray(np.broadcast_to(row, (P, WCOL)))

        m = {
            "x1b": rep(b2[:, 0], pad),
            "x2b": rep(b2[:, 2], pad),
            "y1b": rep(b2[:, 1], pad),
            "y2b": rep(b2[:, 3], pad),
        }
        rows = b1[plan["rowids"][c]].reshape(NT, P, 4)
        a1 = (rows[:, :, 2] - rows[:, :, 0]) * (rows[:, :, 3] - rows[:, :, 1])
        scv = np.empty((P, NT * 5), dtype=np.float32)
        for t in range(NT):
            scv[:, t * 5 + 0] = rows[t, :, 0]
            scv[:, t * 5 + 1] = rows[t, :, 2]
            scv[:, t * 5 + 2] = rows[t, :, 1]
            scv[:, t * 5 + 3] = rows[t, :, 3]
            scv[:, t * 5 + 4] = a1[t]
        m["sc"] = scv
        m["a2e1"] = np.ascontiguousarray(
            np.where(valid, a2e[idxc], np.float32(1.0)).astype(np.float16)
        ).reshape(1, WCOL)
        m["ones1"] = np.ones((1, P), np.float16)
        m["negi"] = (-np.eye(P)).astype(np.float16)
        in_maps.append(m)
    return in_maps


def _assemble(plan, results):
    """Scatter per-tile blocks into the full fp32 matrix."""
    WT, OFFS, bases = plan["WT"], plan["OFFS"], plan["bases"]

    out = np.zeros((N, M), dtype=np.float32)
    for c in range(NCORES):
        blk = np.asarray(results[c]["out"])
        cj = plan["colids"][c]
        rids = plan["rowids"][c]
        for t in range(NT):
            c0 = bases[c] + OFFS[t]
            c1 = c0 + WT[t]
            s0 = max(0, -c0)
            cc0 = max(0, c0)
            cc1 = min(len(cj), c1)
            if cc1 <= cc0:
                continue
            vals = blk[t * P : (t + 1) * P, s0 : s0 + (cc1 - cc0)].astype(
                np.float32
            )
            out[rids[t * P : (t + 1) * P][:, None], cj[cc0:cc1][None, :]] = vals
    return out


def _run(inputs, trace=False, tmpdir=None):
    from concourse.bass_utils import run_bass_kernel_spmd

    plan = _plan(inputs["boxes1"], inputs["boxes2"])
    nc = _get_program(plan["WT"], plan["OFFS"], plan["WCOL"])
    in_maps = _make_in_maps(plan)
    kwargs = {}
    if trace:
        kwargs = dict(trace=True, tmpdir=tmpdir)
    res = run_bass_kernel_spmd(
        nc, in_maps, core_ids=list(range(NCORES)), **kwargs
    )
    return plan, res


def kernel(boxes1: np.ndarray, boxes2: np.ndarray) -> np.ndarray:
    plan, res = _run({"boxes1": boxes1, "boxes2": boxes2})
    return _assemble(plan, res.results)
